# revision 8
# baseline (speedup 1.0000x reference)
"""CommNet message-passing kernel for Trainium2 (8 NeuronCores, data-parallel).

Network (per row r of 131072 = 8192 episodes x 16 agents):
    x  = sigmoid(obs @ enc_w.T + enc_b)
    h  = GRUCell(x, h0)
    2x: c = (sum_agents_in_episode(h) - h) / 16 ; h = GRUCell(c, h)
    weights = h @ dec_w.T + dec_b
    returns (weights, h)

Strategy: shard rows across 8 cores (episodes stay core-local). On-chip,
activations live feature-major ([128 partitions, 2 feature-tiles, N rows]) so
every matmul chains without transposes; the host transposes inputs/outputs.
Matmuls run in float32r (full PE rate, ~1e-4 rounding), gate math in
fp32 on DVE/ACT, gate preactivations accumulate gi+gh in PSUM.
"""

import os
import sys

for _p in ("/opt/trn_rl_repo",):
    if os.path.isdir(_p) and _p not in sys.path:
        sys.path.append(_p)

import numpy as np

import concourse.bass as bass
import concourse.tile as tile
from concourse import bacc, mybir
from concourse.bass_utils import run_bass_kernel_spmd

N_AGENTS = 16
HID = 256
OBS_DIM = 128
N_ACTIONS = 32
K_STEPS = 2
ROWS = 131072
N_CORES = 8
R_CORE = ROWS // N_CORES  # 16384 rows per core

F32 = mybir.dt.float32
F32R = mybir.dt.float32r
AF = mybir.ActivationFunctionType
ALU = mybir.AluOpType


def build_kernel(R: int, N: int):
    """Build the single-core Bass program for R rows, row-tile N."""
    assert R % N == 0 and N % N_AGENTS == 0
    NT = R // N
    NE = N // N_AGENTS  # episodes per tile

    nc = bacc.Bacc("TRN2", target_bir_lowering=False, debug=False)

    # ---- DRAM I/O (feature-major device layouts; host does the transposes) ----
    obs_t = nc.dram_tensor("obs_t", [OBS_DIM, R], F32R, kind="ExternalInput")
    hid_t = nc.dram_tensor("hid_t", [128, 2, R], F32R, kind="ExternalInput")
    enc_w = nc.dram_tensor("enc_w", [128, 256], F32R, kind="ExternalInput")
    wih = nc.dram_tensor("wih", [128, 2, 768], F32R, kind="ExternalInput")
    wihc = nc.dram_tensor("wihc", [128, 2, 768], F32R, kind="ExternalInput")
    whh = nc.dram_tensor("whh", [128, 2, 768], F32R, kind="ExternalInput")
    dec_w = nc.dram_tensor("dec_w", [128, 2, 32], F32R, kind="ExternalInput")
    # biases: cols = [enc0, enc1, r0, r1, z0, z1, ihn0, ihn1, hhn0, hhn1]
    biases = nc.dram_tensor("biases", [128, 10], F32, kind="ExternalInput")
    dec_b = nc.dram_tensor("dec_b", [32, 1], F32, kind="ExternalInput")

    h_out = nc.dram_tensor("h_out", [128, 2, R], F32R, kind="ExternalOutput")
    w_out = nc.dram_tensor("w_out", [32, R], F32, kind="ExternalOutput")

    with tile.TileContext(nc) as tc:
        with (
            tc.tile_pool(name="const", bufs=1) as cpool,
            tc.tile_pool(name="io", bufs=3) as io,
            tc.tile_pool(name="act", bufs=2) as ap,
            tc.tile_pool(name="ps", bufs=8, space="PSUM") as pp,
        ):
            # persistent weights
            wenc = cpool.tile([128, 256], F32R)
            nc.sync.dma_start(wenc[:], enc_w.ap())
            wi = cpool.tile([128, 2, 768], F32R)
            nc.sync.dma_start(wi[:], wih.ap())
            wic = cpool.tile([128, 2, 768], F32R)
            nc.sync.dma_start(wic[:], wihc.ap())
            wh = cpool.tile([128, 2, 768], F32R)
            nc.sync.dma_start(wh[:], whh.ap())
            wd = cpool.tile([128, 2, 32], F32R)
            nc.sync.dma_start(wd[:], dec_w.ap())
            bia = cpool.tile([128, 10], F32)
            nc.sync.dma_start(bia[:], biases.ap())
            bdec = cpool.tile([32, 1], F32)
            nc.sync.dma_start(bdec[:], dec_b.ap())

            def gru(x_in, h_in, wx, n_):
                """One GRUCell; returns the new hidden tile [128,2,N] (f32r).

                All PSUM tiles are single-bank [128, N] allocated round-robin
                from one 8-slot tag so consecutive row-tile iterations overlap.
                """
                r = ap.tile([128, 2, N], F32, tag="r")
                z = ap.tile([128, 2, N], F32, tag="z")
                t = ap.tile([128, 2, N], F32, tag="t")
                u = ap.tile([128, 2, N], F32, tag="u")
                # r,z gates: accumulate gi+gh in PSUM (combined bias via ACT)
                for gate, out, bcol in ((0, r, 2), (1, z, 4)):
                    for m in range(2):
                        col = gate * 256 + m * 128
                        ps = pp.tile([128, N], F32, tag="ps")
                        nc.tensor.matmul(ps[:], wx[:, 0, col:col + 128],
                                         x_in[:, 0, :], start=True, stop=False)
                        nc.tensor.matmul(ps[:], wx[:, 1, col:col + 128],
                                         x_in[:, 1, :], start=False, stop=False)
                        nc.tensor.matmul(ps[:], wh[:, 0, col:col + 128],
                                         h_in[:, 0, :], start=False, stop=False)
                        nc.tensor.matmul(ps[:], wh[:, 1, col:col + 128],
                                         h_in[:, 1, :], start=False, stop=True)
                        nc.scalar.activation(out[:, m, :], ps[:], AF.Sigmoid,
                                             bias=bia[:, bcol + m:bcol + m + 1])
                # n gate: t = (gh_n + b_hhn) * r ; u = (gi_n + b_ihn) + t
                for m in range(2):
                    col = 512 + m * 128
                    ps_nh = pp.tile([128, N], F32, tag="ps")
                    nc.tensor.matmul(ps_nh[:], wh[:, 0, col:col + 128],
                                     h_in[:, 0, :], start=True, stop=False)
                    nc.tensor.matmul(ps_nh[:], wh[:, 1, col:col + 128],
                                     h_in[:, 1, :], start=False, stop=True)
                    nc.vector.scalar_tensor_tensor(
                        t[:, m, :], ps_nh[:], bia[:, 8 + m:9 + m], r[:, m, :],
                        op0=ALU.add, op1=ALU.mult)
                for m in range(2):
                    col = 512 + m * 128
                    ps_ni = pp.tile([128, N], F32, tag="ps")
                    nc.tensor.matmul(ps_ni[:], wx[:, 0, col:col + 128],
                                     x_in[:, 0, :], start=True, stop=False)
                    nc.tensor.matmul(ps_ni[:], wx[:, 1, col:col + 128],
                                     x_in[:, 1, :], start=False, stop=True)
                    nc.vector.scalar_tensor_tensor(
                        u[:, m, :], ps_ni[:], bia[:, 6 + m:7 + m], t[:, m, :],
                        op0=ALU.add, op1=ALU.add)
                nn = ap.tile([128, 2, N], F32, tag="n")
                nc.scalar.activation(nn[:], u[:], AF.Tanh)
                # h' = n + z*(h-n); d,e on GpSimd (idle engine) to unload DVE
                d = ap.tile([128, 2, N], F32, tag="d")
                nc.gpsimd.tensor_tensor(d[:], h_in[:], nn[:], ALU.subtract)
                e = ap.tile([128, 2, N], F32, tag="e")
                nc.gpsimd.tensor_tensor(e[:], z[:], d[:], ALU.mult)
                h2 = ap.tile([128, 2, N], F32R, tag=f"h{n_}")
                nc.vector.tensor_tensor(h2[:], e[:], nn[:], ALU.add)
                return h2

            def comm(h_in):
                """c = (sum over episode agents of h) - h, feature-major."""
                S = ap.tile([128, 2, NE], F32, tag="S")
                h4 = h_in[:].rearrange("p m (e a) -> p m e a", a=N_AGENTS)
                nc.vector.tensor_reduce(S[:], h4, axis=mybir.AxisListType.X,
                                        op=ALU.add)
                c = ap.tile([128, 2, N], F32R, tag="c")
                c4 = c[:].rearrange("p m (e a) -> p m e a", a=N_AGENTS)
                Sb = S[:].broadcast_to([128, 2, NE, N_AGENTS])
                nc.vector.tensor_tensor(c4, Sb, h4, ALU.subtract)
                return c

            # Software pipeline: 4 stages skewed across iterations so the
            # in-order PE always has an independent iteration's matmuls
            # queued while one iteration's gate/comm vector chain drains.
            def stage_a(it):
                sl = slice(it * N, (it + 1) * N)
                obs_s = io.tile([128, N], F32R, tag="obs")
                nc.sync.dma_start(obs_s[:], obs_t.ap()[:, sl])
                h0 = io.tile([128, 2, N], F32R, tag="h0")
                nc.sync.dma_start(h0[:], hid_t.ap()[:, :, sl])
                x = ap.tile([128, 2, N], F32R, tag="x")
                for m in range(2):
                    ps_x = pp.tile([128, N], F32, tag="ps")
                    nc.tensor.matmul(ps_x[:], wenc[:, m * 128:(m + 1) * 128],
                                     obs_s[:], start=True, stop=True)
                    nc.scalar.activation(x[:, m, :], ps_x[:], AF.Sigmoid,
                                         bias=bia[:, m:m + 1])
                return gru(x, h0, wi, 1)

            def stage_b(h1):
                c1 = comm(h1)
                return gru(c1, h1, wic, 2)

            def stage_c(h2):
                c2 = comm(h2)
                return gru(c2, h2, wic, 3)

            def stage_d(it, h3):
                sl = slice(it * N, (it + 1) * N)
                ps_w = pp.tile([128, N], F32, tag="ps")
                nc.tensor.matmul(ps_w[:32, :], wd[:, 0, :], h3[:, 0, :],
                                 start=True, stop=False)
                nc.tensor.matmul(ps_w[:32, :], wd[:, 1, :], h3[:, 1, :],
                                 start=False, stop=True)
                w_s = io.tile([32, N], F32, tag="wout")
                nc.scalar.activation(w_s[:], ps_w[:32, :], AF.Identity,
                                     bias=bdec[:])
                nc.sync.dma_start(h_out.ap()[:, :, sl], h3[:])
                nc.sync.dma_start(w_out.ap()[:, sl], w_s[:])

            h1s, h2s, h3s = {}, {}, {}
            for k in range(NT + 3):
                if k < NT:
                    h1s[k] = stage_a(k)
                if 0 <= k - 1 < NT:
                    h2s[k - 1] = stage_b(h1s.pop(k - 1))
                if 0 <= k - 2 < NT:
                    h3s[k - 2] = stage_c(h2s.pop(k - 2))
                if 0 <= k - 3 < NT:
                    stage_d(k - 3, h3s.pop(k - 3))

    nc.compile()
    return nc


def host_inputs(obs, hidden_state, enc_w, enc_b, w_ih, w_hh, b_ih, b_hh,
                dec_w, dec_b):
    """Shared (replicated) device arrays from the raw weights."""
    f = np.float32
    enc_w_dev = np.ascontiguousarray(enc_w.T, dtype=f)                 # [128,256]
    wih_dev = np.ascontiguousarray(
        w_ih.T.reshape(2, 128, 768).transpose(1, 0, 2), dtype=f)       # [128,2,768]
    wihc_dev = np.ascontiguousarray(wih_dev / N_AGENTS, dtype=f)
    whh_dev = np.ascontiguousarray(
        w_hh.T.reshape(2, 128, 768).transpose(1, 0, 2), dtype=f)
    dec_w_dev = np.ascontiguousarray(
        dec_w.T.reshape(2, 128, 32).transpose(1, 0, 2), dtype=f)       # [128,2,32]

    comb = (b_ih + b_hh).astype(f)
    biases = np.empty((128, 10), dtype=f)
    biases[:, 0:2] = enc_b.reshape(2, 128).T
    biases[:, 2:4] = comb[0:256].reshape(2, 128).T
    biases[:, 4:6] = comb[256:512].reshape(2, 128).T
    biases[:, 6:8] = b_ih[512:768].reshape(2, 128).T
    biases[:, 8:10] = b_hh[512:768].reshape(2, 128).T
    dec_b_dev = np.ascontiguousarray(dec_b.reshape(32, 1), dtype=f)
    return dict(enc_w=enc_w_dev, wih=wih_dev, wihc=wihc_dev, whh=whh_dev,
                dec_w=dec_w_dev, biases=biases, dec_b=dec_b_dev)


def shard_inputs(obs, hidden_state, shared, n_cores=N_CORES):
    R = obs.shape[0] // n_cores
    in_maps = []
    for i in range(n_cores):
        sl = slice(i * R, (i + 1) * R)
        obs_t = np.ascontiguousarray(obs[sl].T, dtype=np.float32)
        hid_t = np.ascontiguousarray(
            hidden_state[sl].reshape(R, 2, 128).transpose(2, 1, 0),
            dtype=np.float32)
        in_maps.append(dict(obs_t=obs_t, hid_t=hid_t, **shared))
    return in_maps


def unshard_outputs(results, R):
    ws, hs = [], []
    for res in results:
        ws.append(np.ascontiguousarray(res["w_out"].T))                # [R,32]
        hs.append(np.ascontiguousarray(
            res["h_out"].transpose(2, 1, 0).reshape(R, 256)))          # [R,256]
    return np.concatenate(ws, axis=0), np.concatenate(hs, axis=0)


_NC_CACHE = {}


def _get_nc(R, N):
    key = (R, N)
    if key not in _NC_CACHE:
        _NC_CACHE[key] = build_kernel(R, N)
    return _NC_CACHE[key]


def kernel(obs, hidden_state, enc_w, enc_b, w_ih, w_hh, b_ih, b_hh,
           dec_w, dec_b):
    nc = _get_nc(R_CORE, 512)
    shared = host_inputs(obs, hidden_state, enc_w, enc_b, w_ih, w_hh,
                         b_ih, b_hh, dec_w, dec_b)
    in_maps = shard_inputs(obs, hidden_state, shared)
    res = run_bass_kernel_spmd(nc, in_maps, list(range(N_CORES)))
    weights, h = unshard_outputs(res.results, R_CORE)
    return weights, h


# revision 10
# speedup vs baseline: 1.1030x; 1.1030x over previous
"""CommNet message-passing kernel for Trainium2 (8 NeuronCores, data-parallel).

Network (per row r of 131072 = 8192 episodes x 16 agents):
    x  = sigmoid(obs @ enc_w.T + enc_b)
    h  = GRUCell(x, h0)
    2x: c = (sum_agents_in_episode(h) - h) / 16 ; h = GRUCell(c, h)
    weights = h @ dec_w.T + dec_b
    returns (weights, h)

Strategy: shard rows across 8 cores (episodes stay core-local). On-chip,
activations live feature-major ([128 partitions, 2 feature-tiles, N rows]) so
every matmul chains without transposes; the host transposes inputs/outputs.
Matmuls run in float32r (full PE rate, ~1e-4 rounding), gate math in
fp32 on DVE/ACT, gate preactivations accumulate gi+gh in PSUM.
"""

import os
import sys

for _p in ("/opt/trn_rl_repo",):
    if os.path.isdir(_p) and _p not in sys.path:
        sys.path.append(_p)

import numpy as np

import concourse.bass as bass
import concourse.tile as tile
from concourse import bacc, mybir
from concourse.bass_utils import run_bass_kernel_spmd

N_AGENTS = 16
HID = 256
OBS_DIM = 128
N_ACTIONS = 32
K_STEPS = 2
ROWS = 131072
N_CORES = 8
R_CORE = ROWS // N_CORES  # 16384 rows per core

F32 = mybir.dt.float32
F32R = mybir.dt.float32r
AF = mybir.ActivationFunctionType
ALU = mybir.AluOpType


def build_kernel(R: int, N: int):
    """Build the single-core Bass program for R rows, row-tile N."""
    assert R % N == 0 and N % N_AGENTS == 0
    NT = R // N
    NE = N // N_AGENTS  # episodes per tile

    nc = bacc.Bacc("TRN2", target_bir_lowering=False, debug=False)

    # ---- DRAM I/O (feature-major device layouts; host does the transposes) ----
    obs_t = nc.dram_tensor("obs_t", [OBS_DIM, R], F32R, kind="ExternalInput")
    hid_t = nc.dram_tensor("hid_t", [128, 2, R], F32R, kind="ExternalInput")
    enc_w = nc.dram_tensor("enc_w", [128, 256], F32R, kind="ExternalInput")
    wih = nc.dram_tensor("wih", [128, 2, 768], F32R, kind="ExternalInput")
    wihc = nc.dram_tensor("wihc", [128, 2, 768], F32R, kind="ExternalInput")
    whh = nc.dram_tensor("whh", [128, 2, 768], F32R, kind="ExternalInput")
    dec_w = nc.dram_tensor("dec_w", [128, 2, 32], F32R, kind="ExternalInput")
    # biases: cols = [enc0, enc1, r0, r1, z0, z1, ihn0, ihn1, hhn0, hhn1]
    biases = nc.dram_tensor("biases", [128, 10], F32, kind="ExternalInput")
    dec_b = nc.dram_tensor("dec_b", [32, 1], F32, kind="ExternalInput")

    h_out = nc.dram_tensor("h_out", [128, 2, R], F32R, kind="ExternalOutput")
    w_out = nc.dram_tensor("w_out", [32, R], F32, kind="ExternalOutput")

    with tile.TileContext(nc) as tc:
        with (
            tc.tile_pool(name="const", bufs=1) as cpool,
            tc.tile_pool(name="io", bufs=3) as io,
            tc.tile_pool(name="act", bufs=2) as ap,
            tc.tile_pool(name="ps", bufs=8, space="PSUM") as pp,
        ):
            # persistent weights
            wenc = cpool.tile([128, 256], F32R)
            nc.sync.dma_start(wenc[:], enc_w.ap())
            wi = cpool.tile([128, 2, 768], F32R)
            nc.sync.dma_start(wi[:], wih.ap())
            wic = cpool.tile([128, 2, 768], F32R)
            nc.sync.dma_start(wic[:], wihc.ap())
            wh = cpool.tile([128, 2, 768], F32R)
            nc.sync.dma_start(wh[:], whh.ap())
            wd = cpool.tile([128, 2, 32], F32R)
            nc.sync.dma_start(wd[:], dec_w.ap())
            bia = cpool.tile([128, 10], F32)
            nc.sync.dma_start(bia[:], biases.ap())
            bdec = cpool.tile([32, 1], F32)
            nc.sync.dma_start(bdec[:], dec_b.ap())

            def gru(x_in, h_in, wx, n_):
                """One GRUCell; returns the new hidden tile [128,2,N] (f32r).

                All PSUM tiles are single-bank [128, N] allocated round-robin
                from one 8-slot tag so consecutive row-tile iterations overlap.
                """
                r = ap.tile([128, 2, N], F32, tag="r")
                z = ap.tile([128, 2, N], F32, tag="z")
                t = ap.tile([128, 2, N], F32, tag="t")
                u = ap.tile([128, 2, N], F32, tag="u")
                # r,z gates: accumulate gi+gh in PSUM (combined bias via ACT)
                for gate, out, bcol in ((0, r, 2), (1, z, 4)):
                    for m in range(2):
                        col = gate * 256 + m * 128
                        ps = pp.tile([128, N], F32, tag="ps")
                        nc.tensor.matmul(ps[:], wx[:, 0, col:col + 128],
                                         x_in[:, 0, :], start=True, stop=False)
                        nc.tensor.matmul(ps[:], wx[:, 1, col:col + 128],
                                         x_in[:, 1, :], start=False, stop=False)
                        nc.tensor.matmul(ps[:], wh[:, 0, col:col + 128],
                                         h_in[:, 0, :], start=False, stop=False)
                        nc.tensor.matmul(ps[:], wh[:, 1, col:col + 128],
                                         h_in[:, 1, :], start=False, stop=True)
                        nc.scalar.activation(out[:, m, :], ps[:], AF.Sigmoid,
                                             bias=bia[:, bcol + m:bcol + m + 1])
                # n gate: t = (gh_n + b_hhn) * r ; u = (gi_n + b_ihn) + t
                for m in range(2):
                    col = 512 + m * 128
                    ps_nh = pp.tile([128, N], F32, tag="ps")
                    nc.tensor.matmul(ps_nh[:], wh[:, 0, col:col + 128],
                                     h_in[:, 0, :], start=True, stop=False)
                    nc.tensor.matmul(ps_nh[:], wh[:, 1, col:col + 128],
                                     h_in[:, 1, :], start=False, stop=True)
                    nc.vector.scalar_tensor_tensor(
                        t[:, m, :], ps_nh[:], bia[:, 8 + m:9 + m], r[:, m, :],
                        op0=ALU.add, op1=ALU.mult)
                for m in range(2):
                    col = 512 + m * 128
                    ps_ni = pp.tile([128, N], F32, tag="ps")
                    nc.tensor.matmul(ps_ni[:], wx[:, 0, col:col + 128],
                                     x_in[:, 0, :], start=True, stop=False)
                    nc.tensor.matmul(ps_ni[:], wx[:, 1, col:col + 128],
                                     x_in[:, 1, :], start=False, stop=True)
                    nc.vector.scalar_tensor_tensor(
                        u[:, m, :], ps_ni[:], bia[:, 6 + m:7 + m], t[:, m, :],
                        op0=ALU.add, op1=ALU.add)
                nn = ap.tile([128, 2, N], F32, tag="n")
                nc.scalar.activation(nn[:], u[:], AF.Tanh)
                # h' = n + z*(h-n)
                d = ap.tile([128, 2, N], F32, tag="d")
                nc.vector.tensor_tensor(d[:], h_in[:], nn[:], ALU.subtract)
                e = ap.tile([128, 2, N], F32, tag="e")
                nc.vector.tensor_tensor(e[:], z[:], d[:], ALU.mult)
                h2 = ap.tile([128, 2, N], F32R, tag=f"h{n_}")
                nc.vector.tensor_tensor(h2[:], e[:], nn[:], ALU.add)
                return h2

            def comm(h_in):
                """c = (sum over episode agents of h) - h, feature-major."""
                S = ap.tile([128, 2, NE], F32, tag="S")
                h4 = h_in[:].rearrange("p m (e a) -> p m e a", a=N_AGENTS)
                nc.vector.tensor_reduce(S[:], h4, axis=mybir.AxisListType.X,
                                        op=ALU.add)
                c = ap.tile([128, 2, N], F32R, tag="c")
                c4 = c[:].rearrange("p m (e a) -> p m e a", a=N_AGENTS)
                Sb = S[:].broadcast_to([128, 2, NE, N_AGENTS])
                nc.gpsimd.tensor_tensor(c4, Sb, h4, ALU.subtract)
                return c

            # Software pipeline: 4 stages skewed across iterations so the
            # in-order PE always has an independent iteration's matmuls
            # queued while one iteration's gate/comm vector chain drains.
            def stage_a(it):
                sl = slice(it * N, (it + 1) * N)
                obs_s = io.tile([128, N], F32R, tag="obs")
                nc.sync.dma_start(obs_s[:], obs_t.ap()[:, sl])
                h0 = io.tile([128, 2, N], F32R, tag="h0")
                nc.sync.dma_start(h0[:], hid_t.ap()[:, :, sl])
                x = ap.tile([128, 2, N], F32R, tag="x")
                for m in range(2):
                    ps_x = pp.tile([128, N], F32, tag="ps")
                    nc.tensor.matmul(ps_x[:], wenc[:, m * 128:(m + 1) * 128],
                                     obs_s[:], start=True, stop=True)
                    nc.scalar.activation(x[:, m, :], ps_x[:], AF.Sigmoid,
                                         bias=bia[:, m:m + 1])
                return gru(x, h0, wi, 1)

            def stage_b(h1):
                c1 = comm(h1)
                return gru(c1, h1, wic, 2)

            def stage_c(h2):
                c2 = comm(h2)
                return gru(c2, h2, wic, 3)

            def stage_d(it, h3):
                sl = slice(it * N, (it + 1) * N)
                ps_w = pp.tile([128, N], F32, tag="ps")
                nc.tensor.matmul(ps_w[:32, :], wd[:, 0, :], h3[:, 0, :],
                                 start=True, stop=False)
                nc.tensor.matmul(ps_w[:32, :], wd[:, 1, :], h3[:, 1, :],
                                 start=False, stop=True)
                w_s = io.tile([32, N], F32, tag="wout")
                nc.scalar.activation(w_s[:], ps_w[:32, :], AF.Identity,
                                     bias=bdec[:])
                nc.sync.dma_start(h_out.ap()[:, :, sl], h3[:])
                nc.sync.dma_start(w_out.ap()[:, sl], w_s[:])

            h1s, h2s, h3s = {}, {}, {}
            for k in range(NT + 3):
                if k < NT:
                    h1s[k] = stage_a(k)
                if 0 <= k - 1 < NT:
                    h2s[k - 1] = stage_b(h1s.pop(k - 1))
                if 0 <= k - 2 < NT:
                    h3s[k - 2] = stage_c(h2s.pop(k - 2))
                if 0 <= k - 3 < NT:
                    stage_d(k - 3, h3s.pop(k - 3))

    nc.compile()
    return nc


def host_inputs(obs, hidden_state, enc_w, enc_b, w_ih, w_hh, b_ih, b_hh,
                dec_w, dec_b):
    """Shared (replicated) device arrays from the raw weights."""
    f = np.float32
    enc_w_dev = np.ascontiguousarray(enc_w.T, dtype=f)                 # [128,256]
    wih_dev = np.ascontiguousarray(
        w_ih.T.reshape(2, 128, 768).transpose(1, 0, 2), dtype=f)       # [128,2,768]
    wihc_dev = np.ascontiguousarray(wih_dev / N_AGENTS, dtype=f)
    whh_dev = np.ascontiguousarray(
        w_hh.T.reshape(2, 128, 768).transpose(1, 0, 2), dtype=f)
    dec_w_dev = np.ascontiguousarray(
        dec_w.T.reshape(2, 128, 32).transpose(1, 0, 2), dtype=f)       # [128,2,32]

    comb = (b_ih + b_hh).astype(f)
    biases = np.empty((128, 10), dtype=f)
    biases[:, 0:2] = enc_b.reshape(2, 128).T
    biases[:, 2:4] = comb[0:256].reshape(2, 128).T
    biases[:, 4:6] = comb[256:512].reshape(2, 128).T
    biases[:, 6:8] = b_ih[512:768].reshape(2, 128).T
    biases[:, 8:10] = b_hh[512:768].reshape(2, 128).T
    dec_b_dev = np.ascontiguousarray(dec_b.reshape(32, 1), dtype=f)
    return dict(enc_w=enc_w_dev, wih=wih_dev, wihc=wihc_dev, whh=whh_dev,
                dec_w=dec_w_dev, biases=biases, dec_b=dec_b_dev)


def shard_inputs(obs, hidden_state, shared, n_cores=N_CORES):
    R = obs.shape[0] // n_cores
    in_maps = []
    for i in range(n_cores):
        sl = slice(i * R, (i + 1) * R)
        obs_t = np.ascontiguousarray(obs[sl].T, dtype=np.float32)
        hid_t = np.ascontiguousarray(
            hidden_state[sl].reshape(R, 2, 128).transpose(2, 1, 0),
            dtype=np.float32)
        in_maps.append(dict(obs_t=obs_t, hid_t=hid_t, **shared))
    return in_maps


def unshard_outputs(results, R):
    ws, hs = [], []
    for res in results:
        ws.append(np.ascontiguousarray(res["w_out"].T))                # [R,32]
        hs.append(np.ascontiguousarray(
            res["h_out"].transpose(2, 1, 0).reshape(R, 256)))          # [R,256]
    return np.concatenate(ws, axis=0), np.concatenate(hs, axis=0)


_NC_CACHE = {}


def _get_nc(R, N):
    key = (R, N)
    if key not in _NC_CACHE:
        _NC_CACHE[key] = build_kernel(R, N)
    return _NC_CACHE[key]


def kernel(obs, hidden_state, enc_w, enc_b, w_ih, w_hh, b_ih, b_hh,
           dec_w, dec_b):
    nc = _get_nc(R_CORE, 512)
    shared = host_inputs(obs, hidden_state, enc_w, enc_b, w_ih, w_hh,
                         b_ih, b_hh, dec_w, dec_b)
    in_maps = shard_inputs(obs, hidden_state, shared)
    res = run_bass_kernel_spmd(nc, in_maps, list(range(N_CORES)))
    weights, h = unshard_outputs(res.results, R_CORE)
    return weights, h


# revision 11
# speedup vs baseline: 1.2993x; 1.1779x over previous
"""CommNet message-passing kernel for Trainium2 (8 NeuronCores, data-parallel).

Network (per row r of 131072 = 8192 episodes x 16 agents):
    x  = sigmoid(obs @ enc_w.T + enc_b)
    h  = GRUCell(x, h0)
    2x: c = (sum_agents_in_episode(h) - h) / 16 ; h = GRUCell(c, h)
    weights = h @ dec_w.T + dec_b
    returns (weights, h)

Strategy: shard rows across 8 cores (episodes stay core-local). On-chip,
activations live feature-major ([128 partitions, 2 feature-tiles, N rows]) so
every matmul chains without transposes; the host transposes inputs/outputs.
Matmuls run in float32r (full PE rate, ~1e-4 rounding), gate math in
fp32 on DVE/ACT, gate preactivations accumulate gi+gh in PSUM.
"""

import os
import sys

for _p in ("/opt/trn_rl_repo",):
    if os.path.isdir(_p) and _p not in sys.path:
        sys.path.append(_p)

import numpy as np

import concourse.bass as bass
import concourse.tile as tile
from concourse import bacc, mybir
from concourse.bass_utils import run_bass_kernel_spmd

N_AGENTS = 16
HID = 256
OBS_DIM = 128
N_ACTIONS = 32
K_STEPS = 2
ROWS = 131072
N_CORES = 8
R_CORE = ROWS // N_CORES  # 16384 rows per core

F32 = mybir.dt.float32
F32R = mybir.dt.float32r
F16 = mybir.dt.float16
AF = mybir.ActivationFunctionType
ALU = mybir.AluOpType

# activation-pipeline precision: "f32r" (safest) or "fp16" (2x DVE modes)
PREC = "fp16"
DT_MM = F16 if PREC == "fp16" else F32R    # matmul-operand / h dtype
DT_EW = F16 if PREC == "fp16" else F32     # gate elementwise dtype
NP_IN = "float16" if PREC == "fp16" else "float32"


def build_kernel(R: int, N: int):
    """Build the single-core Bass program for R rows, row-tile N."""
    assert R % N == 0 and N % N_AGENTS == 0
    NT = R // N
    NE = N // N_AGENTS  # episodes per tile

    nc = bacc.Bacc("TRN2", target_bir_lowering=False, debug=False)

    # ---- DRAM I/O (feature-major device layouts; host does the transposes) ----
    obs_t = nc.dram_tensor("obs_t", [OBS_DIM, R], DT_MM, kind="ExternalInput")
    hid_t = nc.dram_tensor("hid_t", [128, 2, R], DT_MM, kind="ExternalInput")
    enc_w = nc.dram_tensor("enc_w", [128, 256], DT_MM, kind="ExternalInput")
    wih = nc.dram_tensor("wih", [128, 2, 768], DT_MM, kind="ExternalInput")
    wihc = nc.dram_tensor("wihc", [128, 2, 768], DT_MM, kind="ExternalInput")
    whh = nc.dram_tensor("whh", [128, 2, 768], DT_MM, kind="ExternalInput")
    dec_w = nc.dram_tensor("dec_w", [128, 2, 32], DT_MM, kind="ExternalInput")
    # biases: cols = [enc0, enc1, r0, r1, z0, z1, ihn0, ihn1, hhn0, hhn1]
    biases = nc.dram_tensor("biases", [128, 10], F32, kind="ExternalInput")
    dec_b = nc.dram_tensor("dec_b", [32, 1], F32, kind="ExternalInput")

    h_out = nc.dram_tensor("h_out", [128, 2, R], DT_MM, kind="ExternalOutput")
    w_out = nc.dram_tensor("w_out", [32, R], F32, kind="ExternalOutput")

    with tile.TileContext(nc) as tc:
        with (
            tc.tile_pool(name="const", bufs=1) as cpool,
            tc.tile_pool(name="io", bufs=3) as io,
            tc.tile_pool(name="act", bufs=2) as ap,
            tc.tile_pool(name="ps", bufs=8, space="PSUM") as pp,
        ):
            # persistent weights
            wenc = cpool.tile([128, 256], DT_MM)
            nc.sync.dma_start(wenc[:], enc_w.ap())
            wi = cpool.tile([128, 2, 768], DT_MM)
            nc.sync.dma_start(wi[:], wih.ap())
            wic = cpool.tile([128, 2, 768], DT_MM)
            nc.sync.dma_start(wic[:], wihc.ap())
            wh = cpool.tile([128, 2, 768], DT_MM)
            nc.sync.dma_start(wh[:], whh.ap())
            wd = cpool.tile([128, 2, 32], DT_MM)
            nc.sync.dma_start(wd[:], dec_w.ap())
            bia = cpool.tile([128, 10], F32)
            nc.sync.dma_start(bia[:], biases.ap())
            bdec = cpool.tile([32, 1], F32)
            nc.sync.dma_start(bdec[:], dec_b.ap())

            def gru(x_in, h_in, wx, n_):
                """One GRUCell; returns the new hidden tile [128,2,N] (f32r).

                All PSUM tiles are single-bank [128, N] allocated round-robin
                from one 8-slot tag so consecutive row-tile iterations overlap.
                """
                r = ap.tile([128, 2, N], DT_EW, tag="r")
                z = ap.tile([128, 2, N], DT_EW, tag="z")
                t = ap.tile([128, 2, N], DT_EW, tag="t")
                u = ap.tile([128, 2, N], DT_EW, tag="u")
                # r,z gates: accumulate gi+gh in PSUM (combined bias via ACT)
                for gate, out, bcol in ((0, r, 2), (1, z, 4)):
                    for m in range(2):
                        col = gate * 256 + m * 128
                        ps = pp.tile([128, N], F32, tag="ps")
                        nc.tensor.matmul(ps[:], wx[:, 0, col:col + 128],
                                         x_in[:, 0, :], start=True, stop=False)
                        nc.tensor.matmul(ps[:], wx[:, 1, col:col + 128],
                                         x_in[:, 1, :], start=False, stop=False)
                        nc.tensor.matmul(ps[:], wh[:, 0, col:col + 128],
                                         h_in[:, 0, :], start=False, stop=False)
                        nc.tensor.matmul(ps[:], wh[:, 1, col:col + 128],
                                         h_in[:, 1, :], start=False, stop=True)
                        nc.scalar.activation(out[:, m, :], ps[:], AF.Sigmoid,
                                             bias=bia[:, bcol + m:bcol + m + 1])
                # n gate: t = (gh_n + b_hhn) * r ; u = (gi_n + b_ihn) + t
                for m in range(2):
                    col = 512 + m * 128
                    ps_nh = pp.tile([128, N], F32, tag="ps")
                    nc.tensor.matmul(ps_nh[:], wh[:, 0, col:col + 128],
                                     h_in[:, 0, :], start=True, stop=False)
                    nc.tensor.matmul(ps_nh[:], wh[:, 1, col:col + 128],
                                     h_in[:, 1, :], start=False, stop=True)
                    nc.vector.scalar_tensor_tensor(
                        t[:, m, :], ps_nh[:], bia[:, 8 + m:9 + m], r[:, m, :],
                        op0=ALU.add, op1=ALU.mult)
                for m in range(2):
                    col = 512 + m * 128
                    ps_ni = pp.tile([128, N], F32, tag="ps")
                    nc.tensor.matmul(ps_ni[:], wx[:, 0, col:col + 128],
                                     x_in[:, 0, :], start=True, stop=False)
                    nc.tensor.matmul(ps_ni[:], wx[:, 1, col:col + 128],
                                     x_in[:, 1, :], start=False, stop=True)
                    nc.vector.scalar_tensor_tensor(
                        u[:, m, :], ps_ni[:], bia[:, 6 + m:7 + m], t[:, m, :],
                        op0=ALU.add, op1=ALU.add)
                nn = ap.tile([128, 2, N], DT_EW, tag="n")
                nc.scalar.activation(nn[:], u[:], AF.Tanh)
                # h' = n + z*(h-n)
                d = ap.tile([128, 2, N], DT_EW, tag="d")
                nc.vector.tensor_tensor(d[:], h_in[:], nn[:], ALU.subtract)
                e = ap.tile([128, 2, N], DT_EW, tag="e")
                nc.vector.tensor_tensor(e[:], z[:], d[:], ALU.mult)
                h2 = ap.tile([128, 2, N], DT_MM, tag=f"h{n_}")
                nc.vector.tensor_tensor(h2[:], e[:], nn[:], ALU.add)
                return h2

            def comm(h_in):
                """c = (sum over episode agents of h) - h, feature-major."""
                S = ap.tile([128, 2, NE], F32, tag="S")
                h4 = h_in[:].rearrange("p m (e a) -> p m e a", a=N_AGENTS)
                nc.vector.tensor_reduce(S[:], h4, axis=mybir.AxisListType.X,
                                        op=ALU.add)
                c = ap.tile([128, 2, N], DT_MM, tag="c")
                c4 = c[:].rearrange("p m (e a) -> p m e a", a=N_AGENTS)
                Sb = S[:].broadcast_to([128, 2, NE, N_AGENTS])
                nc.gpsimd.tensor_tensor(c4, Sb, h4, ALU.subtract)
                return c

            # Software pipeline: 4 stages skewed across iterations so the
            # in-order PE always has an independent iteration's matmuls
            # queued while one iteration's gate/comm vector chain drains.
            def stage_a(it):
                sl = slice(it * N, (it + 1) * N)
                obs_s = io.tile([128, N], DT_MM, tag="obs")
                nc.sync.dma_start(obs_s[:], obs_t.ap()[:, sl])
                h0 = io.tile([128, 2, N], DT_MM, tag="h0")
                nc.sync.dma_start(h0[:], hid_t.ap()[:, :, sl])
                x = ap.tile([128, 2, N], DT_MM, tag="x")
                for m in range(2):
                    ps_x = pp.tile([128, N], F32, tag="ps")
                    nc.tensor.matmul(ps_x[:], wenc[:, m * 128:(m + 1) * 128],
                                     obs_s[:], start=True, stop=True)
                    nc.scalar.activation(x[:, m, :], ps_x[:], AF.Sigmoid,
                                         bias=bia[:, m:m + 1])
                return gru(x, h0, wi, 1)

            def stage_b(h1):
                c1 = comm(h1)
                return gru(c1, h1, wic, 2)

            def stage_c(h2):
                c2 = comm(h2)
                return gru(c2, h2, wic, 3)

            def stage_d(it, h3):
                sl = slice(it * N, (it + 1) * N)
                ps_w = pp.tile([128, N], F32, tag="ps")
                nc.tensor.matmul(ps_w[:32, :], wd[:, 0, :], h3[:, 0, :],
                                 start=True, stop=False)
                nc.tensor.matmul(ps_w[:32, :], wd[:, 1, :], h3[:, 1, :],
                                 start=False, stop=True)
                w_s = io.tile([32, N], F32, tag="wout")
                nc.scalar.activation(w_s[:], ps_w[:32, :], AF.Identity,
                                     bias=bdec[:])
                nc.sync.dma_start(h_out.ap()[:, :, sl], h3[:])
                nc.sync.dma_start(w_out.ap()[:, sl], w_s[:])

            h1s, h2s, h3s = {}, {}, {}
            for k in range(NT + 3):
                if k < NT:
                    h1s[k] = stage_a(k)
                if 0 <= k - 1 < NT:
                    h2s[k - 1] = stage_b(h1s.pop(k - 1))
                if 0 <= k - 2 < NT:
                    h3s[k - 2] = stage_c(h2s.pop(k - 2))
                if 0 <= k - 3 < NT:
                    stage_d(k - 3, h3s.pop(k - 3))

    nc.compile()
    return nc


def host_inputs(obs, hidden_state, enc_w, enc_b, w_ih, w_hh, b_ih, b_hh,
                dec_w, dec_b):
    """Shared (replicated) device arrays from the raw weights."""
    f = np.dtype(NP_IN)
    enc_w_dev = np.ascontiguousarray(enc_w.T, dtype=f)                 # [128,256]
    wih_dev = np.ascontiguousarray(
        w_ih.T.reshape(2, 128, 768).transpose(1, 0, 2), dtype=f)       # [128,2,768]
    wihc_dev = np.ascontiguousarray(wih_dev / N_AGENTS, dtype=f)
    whh_dev = np.ascontiguousarray(
        w_hh.T.reshape(2, 128, 768).transpose(1, 0, 2), dtype=f)
    dec_w_dev = np.ascontiguousarray(
        dec_w.T.reshape(2, 128, 32).transpose(1, 0, 2), dtype=f)       # [128,2,32]

    comb = (b_ih + b_hh).astype(np.float32)
    biases = np.empty((128, 10), dtype=np.float32)
    biases[:, 0:2] = enc_b.reshape(2, 128).T
    biases[:, 2:4] = comb[0:256].reshape(2, 128).T
    biases[:, 4:6] = comb[256:512].reshape(2, 128).T
    biases[:, 6:8] = b_ih[512:768].reshape(2, 128).T
    biases[:, 8:10] = b_hh[512:768].reshape(2, 128).T
    dec_b_dev = np.ascontiguousarray(dec_b.reshape(32, 1), dtype=np.float32)
    return dict(enc_w=enc_w_dev, wih=wih_dev, wihc=wihc_dev, whh=whh_dev,
                dec_w=dec_w_dev, biases=biases, dec_b=dec_b_dev)


def shard_inputs(obs, hidden_state, shared, n_cores=N_CORES):
    R = obs.shape[0] // n_cores
    in_maps = []
    for i in range(n_cores):
        sl = slice(i * R, (i + 1) * R)
        obs_t = np.ascontiguousarray(obs[sl].T, dtype=np.dtype(NP_IN))
        hid_t = np.ascontiguousarray(
            hidden_state[sl].reshape(R, 2, 128).transpose(2, 1, 0),
            dtype=np.dtype(NP_IN))
        in_maps.append(dict(obs_t=obs_t, hid_t=hid_t, **shared))
    return in_maps


def unshard_outputs(results, R):
    ws, hs = [], []
    for res in results:
        ws.append(np.ascontiguousarray(res["w_out"].T))                # [R,32]
        hs.append(np.ascontiguousarray(
            res["h_out"].transpose(2, 1, 0).reshape(R, 256).astype(np.float32)))          # [R,256]
    return np.concatenate(ws, axis=0), np.concatenate(hs, axis=0)


_NC_CACHE = {}


def _get_nc(R, N):
    key = (R, N)
    if key not in _NC_CACHE:
        _NC_CACHE[key] = build_kernel(R, N)
    return _NC_CACHE[key]


def kernel(obs, hidden_state, enc_w, enc_b, w_ih, w_hh, b_ih, b_hh,
           dec_w, dec_b):
    nc = _get_nc(R_CORE, 512)
    shared = host_inputs(obs, hidden_state, enc_w, enc_b, w_ih, w_hh,
                         b_ih, b_hh, dec_w, dec_b)
    in_maps = shard_inputs(obs, hidden_state, shared)
    res = run_bass_kernel_spmd(nc, in_maps, list(range(N_CORES)))
    weights, h = unshard_outputs(res.results, R_CORE)
    return weights, h


# revision 13
# speedup vs baseline: 1.4422x; 1.1100x over previous
"""CommNet message-passing kernel for Trainium2 (8 NeuronCores, data-parallel).

Network (per row r of 131072 = 8192 episodes x 16 agents):
    x  = sigmoid(obs @ enc_w.T + enc_b)
    h  = GRUCell(x, h0)
    2x: c = (sum_agents_in_episode(h) - h) / 16 ; h = GRUCell(c, h)
    weights = h @ dec_w.T + dec_b
    returns (weights, h)

Strategy: shard rows across 8 cores (episodes stay core-local). On-chip,
activations live feature-major ([128 partitions, 2 feature-tiles, N rows]) so
every matmul chains without transposes; the host transposes inputs/outputs.
Matmuls run in float32r (full PE rate, ~1e-4 rounding), gate math in
fp32 on DVE/ACT, gate preactivations accumulate gi+gh in PSUM.
"""

import os
import sys

for _p in ("/opt/trn_rl_repo",):
    if os.path.isdir(_p) and _p not in sys.path:
        sys.path.append(_p)

import numpy as np

import concourse.bass as bass
import concourse.tile as tile
from concourse import bacc, mybir
from concourse.bass_utils import run_bass_kernel_spmd

N_AGENTS = 16
HID = 256
OBS_DIM = 128
N_ACTIONS = 32
K_STEPS = 2
ROWS = 131072
N_CORES = 8
R_CORE = ROWS // N_CORES  # 16384 rows per core

F32 = mybir.dt.float32
F32R = mybir.dt.float32r
F16 = mybir.dt.float16
AF = mybir.ActivationFunctionType
ALU = mybir.AluOpType

# activation-pipeline precision: "f32r" (safest) or "fp16" (2x DVE modes)
PREC = "fp16"
DT_MM = F16 if PREC == "fp16" else F32R    # matmul-operand / h dtype
DT_EW = F16 if PREC == "fp16" else F32     # gate elementwise dtype
NP_IN = "float16" if PREC == "fp16" else "float32"


def build_kernel(R: int, N: int):
    """Build the single-core Bass program for R rows, row-tile N."""
    assert R % N == 0 and N % N_AGENTS == 0
    NT = R // N
    NE = N // N_AGENTS  # episodes per tile

    nc = bacc.Bacc("TRN2", target_bir_lowering=False, debug=False)

    # ---- DRAM I/O (feature-major device layouts; host does the transposes) ----
    obs_t = nc.dram_tensor("obs_t", [OBS_DIM, R], DT_MM, kind="ExternalInput")
    hid_t = nc.dram_tensor("hid_t", [128, 2, R], DT_MM, kind="ExternalInput")
    enc_w = nc.dram_tensor("enc_w", [128, 256], DT_MM, kind="ExternalInput")
    wih = nc.dram_tensor("wih", [128, 2, 768], DT_MM, kind="ExternalInput")
    wihc = nc.dram_tensor("wihc", [128, 2, 768], DT_MM, kind="ExternalInput")
    whh = nc.dram_tensor("whh", [128, 2, 768], DT_MM, kind="ExternalInput")
    dec_w = nc.dram_tensor("dec_w", [128, 2, 32], DT_MM, kind="ExternalInput")
    # biases: cols = [enc0, enc1, r0, r1, z0, z1, ihn0, ihn1, hhn0, hhn1]
    biases = nc.dram_tensor("biases", [128, 10], F32, kind="ExternalInput")
    dec_b = nc.dram_tensor("dec_b", [32, 1], F32, kind="ExternalInput")

    h_out = nc.dram_tensor("h_out", [128, 2, R], DT_MM, kind="ExternalOutput")
    w_out = nc.dram_tensor("w_out", [32, R], F32, kind="ExternalOutput")

    with tile.TileContext(nc) as tc:
        with (
            tc.tile_pool(name="const", bufs=1) as cpool,
            tc.tile_pool(name="io", bufs=3) as io,
            tc.tile_pool(name="act", bufs=2) as ap,
            tc.tile_pool(name="ps", bufs=8, space="PSUM") as pp,
        ):
            # persistent weights
            wenc = cpool.tile([128, 256], DT_MM)
            nc.sync.dma_start(wenc[:], enc_w.ap())
            wi = cpool.tile([128, 2, 768], DT_MM)
            nc.sync.dma_start(wi[:], wih.ap())
            wic = cpool.tile([128, 2, 768], DT_MM)
            nc.sync.dma_start(wic[:], wihc.ap())
            wh = cpool.tile([128, 2, 768], DT_MM)
            nc.sync.dma_start(wh[:], whh.ap())
            wd = cpool.tile([128, 2, 32], DT_MM)
            nc.sync.dma_start(wd[:], dec_w.ap())
            bia = cpool.tile([128, 10], F32)
            nc.sync.dma_start(bia[:], biases.ap())
            bdec = cpool.tile([32, 1], F32)
            nc.sync.dma_start(bdec[:], dec_b.ap())

            def gru(x_in, h_in, wx, n_):
                """One GRUCell; returns the new hidden tile [128,2,N] (f32r).

                All PSUM tiles are single-bank [128, N] allocated round-robin
                from one 8-slot tag so consecutive row-tile iterations overlap.
                """
                r = ap.tile([128, 2, N], DT_EW, tag="r")
                z = ap.tile([128, 2, N], DT_EW, tag="z")
                t = ap.tile([128, 2, N], DT_EW, tag="t")
                u = ap.tile([128, 2, N], DT_EW, tag="u")
                # r,z gates: accumulate gi+gh in PSUM (combined bias via ACT)
                for gate, out, bcol in ((0, r, 2), (1, z, 4)):
                    for m in range(2):
                        col = gate * 256 + m * 128
                        ps = pp.tile([128, N], F32, tag="ps")
                        nc.tensor.matmul(ps[:], wx[:, 0, col:col + 128],
                                         x_in[:, 0, :], start=True, stop=False)
                        nc.tensor.matmul(ps[:], wx[:, 1, col:col + 128],
                                         x_in[:, 1, :], start=False, stop=False)
                        nc.tensor.matmul(ps[:], wh[:, 0, col:col + 128],
                                         h_in[:, 0, :], start=False, stop=False)
                        nc.tensor.matmul(ps[:], wh[:, 1, col:col + 128],
                                         h_in[:, 1, :], start=False, stop=True)
                        nc.scalar.activation(out[:, m, :], ps[:], AF.Sigmoid,
                                             bias=bia[:, bcol + m:bcol + m + 1])
                # n gate: gh_n' = gh_n + b_hhn (ACT Identity, psum->fp16 SBUF)
                # then t = gh_n' * r (2x fp16 TT); u = (gi_n + b_ihn) + t (STT)
                ghn = ap.tile([128, 2, N], DT_EW, tag="ghn")
                for m in range(2):
                    col = 512 + m * 128
                    ps_nh = pp.tile([128, N], F32, tag="ps")
                    nc.tensor.matmul(ps_nh[:], wh[:, 0, col:col + 128],
                                     h_in[:, 0, :], start=True, stop=False)
                    nc.tensor.matmul(ps_nh[:], wh[:, 1, col:col + 128],
                                     h_in[:, 1, :], start=False, stop=True)
                    nc.scalar.activation(ghn[:, m, :], ps_nh[:], AF.Identity,
                                         bias=bia[:, 8 + m:9 + m])
                nc.vector.tensor_tensor(t[:], ghn[:], r[:], ALU.mult)
                for m in range(2):
                    col = 512 + m * 128
                    ps_ni = pp.tile([128, N], F32, tag="ps")
                    nc.tensor.matmul(ps_ni[:], wx[:, 0, col:col + 128],
                                     x_in[:, 0, :], start=True, stop=False)
                    nc.tensor.matmul(ps_ni[:], wx[:, 1, col:col + 128],
                                     x_in[:, 1, :], start=False, stop=True)
                    nc.vector.scalar_tensor_tensor(
                        u[:, m, :], ps_ni[:], bia[:, 6 + m:7 + m], t[:, m, :],
                        op0=ALU.add, op1=ALU.add)
                nn = ap.tile([128, 2, N], DT_EW, tag="n")
                nc.scalar.activation(nn[:], u[:], AF.Tanh)
                # h' = n + z*(h-n)
                d = ap.tile([128, 2, N], DT_EW, tag="d")
                nc.vector.tensor_tensor(d[:], h_in[:], nn[:], ALU.subtract)
                e = ap.tile([128, 2, N], DT_EW, tag="e")
                nc.vector.tensor_tensor(e[:], z[:], d[:], ALU.mult)
                h2 = ap.tile([128, 2, N], DT_MM, tag=f"h{n_}")
                nc.vector.tensor_tensor(h2[:], e[:], nn[:], ALU.add)
                return h2

            def comm(h_in):
                """c = (sum over episode agents of h) - h, feature-major."""
                S = ap.tile([128, 2, NE], F32, tag="S")
                h4 = h_in[:].rearrange("p m (e a) -> p m e a", a=N_AGENTS)
                nc.vector.tensor_reduce(S[:], h4, axis=mybir.AxisListType.X,
                                        op=ALU.add)
                c = ap.tile([128, 2, N], DT_MM, tag="c")
                c4 = c[:].rearrange("p m (e a) -> p m e a", a=N_AGENTS)
                Sb = S[:].broadcast_to([128, 2, NE, N_AGENTS])
                nc.vector.tensor_tensor(c4, Sb, h4, ALU.subtract)
                return c

            # Software pipeline: 4 stages skewed across iterations so the
            # in-order PE always has an independent iteration's matmuls
            # queued while one iteration's gate/comm vector chain drains.
            def stage_a(it):
                sl = slice(it * N, (it + 1) * N)
                obs_s = io.tile([128, N], DT_MM, tag="obs")
                nc.sync.dma_start(obs_s[:], obs_t.ap()[:, sl])
                h0 = io.tile([128, 2, N], DT_MM, tag="h0")
                nc.sync.dma_start(h0[:], hid_t.ap()[:, :, sl])
                x = ap.tile([128, 2, N], DT_MM, tag="x")
                for m in range(2):
                    ps_x = pp.tile([128, N], F32, tag="ps")
                    nc.tensor.matmul(ps_x[:], wenc[:, m * 128:(m + 1) * 128],
                                     obs_s[:], start=True, stop=True)
                    nc.scalar.activation(x[:, m, :], ps_x[:], AF.Sigmoid,
                                         bias=bia[:, m:m + 1])
                return gru(x, h0, wi, 1)

            def stage_b(h1):
                c1 = comm(h1)
                return gru(c1, h1, wic, 2)

            def stage_c(h2):
                c2 = comm(h2)
                return gru(c2, h2, wic, 3)

            def stage_d(it, h3):
                sl = slice(it * N, (it + 1) * N)
                ps_w = pp.tile([128, N], F32, tag="ps")
                nc.tensor.matmul(ps_w[:32, :], wd[:, 0, :], h3[:, 0, :],
                                 start=True, stop=False)
                nc.tensor.matmul(ps_w[:32, :], wd[:, 1, :], h3[:, 1, :],
                                 start=False, stop=True)
                w_s = io.tile([32, N], F32, tag="wout")
                nc.scalar.activation(w_s[:], ps_w[:32, :], AF.Identity,
                                     bias=bdec[:])
                nc.sync.dma_start(h_out.ap()[:, :, sl], h3[:])
                nc.sync.dma_start(w_out.ap()[:, sl], w_s[:])

            h1s, h2s, h3s = {}, {}, {}
            for k in range(NT + 3):
                if k < NT:
                    h1s[k] = stage_a(k)
                if 0 <= k - 1 < NT:
                    h2s[k - 1] = stage_b(h1s.pop(k - 1))
                if 0 <= k - 2 < NT:
                    h3s[k - 2] = stage_c(h2s.pop(k - 2))
                if 0 <= k - 3 < NT:
                    stage_d(k - 3, h3s.pop(k - 3))

    nc.compile()
    return nc


def host_inputs(obs, hidden_state, enc_w, enc_b, w_ih, w_hh, b_ih, b_hh,
                dec_w, dec_b):
    """Shared (replicated) device arrays from the raw weights."""
    f = np.dtype(NP_IN)
    enc_w_dev = np.ascontiguousarray(enc_w.T, dtype=f)                 # [128,256]
    wih_dev = np.ascontiguousarray(
        w_ih.T.reshape(2, 128, 768).transpose(1, 0, 2), dtype=f)       # [128,2,768]
    wihc_dev = np.ascontiguousarray(wih_dev / N_AGENTS, dtype=f)
    whh_dev = np.ascontiguousarray(
        w_hh.T.reshape(2, 128, 768).transpose(1, 0, 2), dtype=f)
    dec_w_dev = np.ascontiguousarray(
        dec_w.T.reshape(2, 128, 32).transpose(1, 0, 2), dtype=f)       # [128,2,32]

    comb = (b_ih + b_hh).astype(np.float32)
    biases = np.empty((128, 10), dtype=np.float32)
    biases[:, 0:2] = enc_b.reshape(2, 128).T
    biases[:, 2:4] = comb[0:256].reshape(2, 128).T
    biases[:, 4:6] = comb[256:512].reshape(2, 128).T
    biases[:, 6:8] = b_ih[512:768].reshape(2, 128).T
    biases[:, 8:10] = b_hh[512:768].reshape(2, 128).T
    dec_b_dev = np.ascontiguousarray(dec_b.reshape(32, 1), dtype=np.float32)
    return dict(enc_w=enc_w_dev, wih=wih_dev, wihc=wihc_dev, whh=whh_dev,
                dec_w=dec_w_dev, biases=biases, dec_b=dec_b_dev)


def shard_inputs(obs, hidden_state, shared, n_cores=N_CORES):
    R = obs.shape[0] // n_cores
    in_maps = []
    for i in range(n_cores):
        sl = slice(i * R, (i + 1) * R)
        obs_t = np.ascontiguousarray(obs[sl].T, dtype=np.dtype(NP_IN))
        hid_t = np.ascontiguousarray(
            hidden_state[sl].reshape(R, 2, 128).transpose(2, 1, 0),
            dtype=np.dtype(NP_IN))
        in_maps.append(dict(obs_t=obs_t, hid_t=hid_t, **shared))
    return in_maps


def unshard_outputs(results, R):
    ws, hs = [], []
    for res in results:
        ws.append(np.ascontiguousarray(res["w_out"].T))                # [R,32]
        hs.append(np.ascontiguousarray(
            res["h_out"].transpose(2, 1, 0).reshape(R, 256).astype(np.float32)))          # [R,256]
    return np.concatenate(ws, axis=0), np.concatenate(hs, axis=0)


_NC_CACHE = {}


def _get_nc(R, N):
    key = (R, N)
    if key not in _NC_CACHE:
        _NC_CACHE[key] = build_kernel(R, N)
    return _NC_CACHE[key]


def kernel(obs, hidden_state, enc_w, enc_b, w_ih, w_hh, b_ih, b_hh,
           dec_w, dec_b):
    nc = _get_nc(R_CORE, 512)
    shared = host_inputs(obs, hidden_state, enc_w, enc_b, w_ih, w_hh,
                         b_ih, b_hh, dec_w, dec_b)
    in_maps = shard_inputs(obs, hidden_state, shared)
    res = run_bass_kernel_spmd(nc, in_maps, list(range(N_CORES)))
    weights, h = unshard_outputs(res.results, R_CORE)
    return weights, h
